# revision 8
# baseline (speedup 1.0000x reference)
"""BinaryConv2D Trainium2 kernel — 1D Winograd F(2,3) along image width.

Full computation:
  out = conv2d(sign(pad(x)), sign(k)) * avgpool3x3(mean|pad(x)|_ci) * alpha + bias

The 3x3 conv is computed as a vertical-direct x horizontal-Winograd hybrid:
per output column pair (2c, 2c+1), F(2,3) gives 4 products m1..m4 from the
transformed inputs V1..V4 (values in {0,+-2}, exact in fp8) and transformed
weights U1..U4 (values in {+-0.5,+-1.5,+-1}, exact in fp8):

  y_even = m1 + m2 + m3        y_odd = m2 - m3 - m4

The vertical 3 taps stay direct: each m_i accumulates 3 row-shifted matmuls
in PSUM (fp8 DoubleRow, 256-wide contraction).  To minimize both PE streams
and DVE combine work, the comps are split over 3 PSUM banks:

  B1 = m2 (3 MMs)   B2 = m1 + m3 (6 MMs)   B3 = -m3 - m4 (6 MMs)
  y_even = B1 + B2             y_odd = B1 + B3

15 tap-instance streams per column pair replace direct conv's 18 -> the
tensor-engine stream drops from ~105us to ~78us per core.  Epilogue per
group: ScalarE drains B1 to SBUF; DVE does two fused scalar_tensor_tensor
ops per parity ((B add sb), then (t*alpha)*K); ScalarE adds bias.
Everything is exact integer/quarter arithmetic until the K*alpha scaling.

Device strategy: 8 NeuronCores, data-parallel over batch N=32 -> 4 img/core.
"""

import sys

for _p in ("/root/.axon_site/_ro/trn_rl_repo", "/opt/trn_rl_repo"):
    if _p not in sys.path:
        sys.path.append(_p)

import numpy as np
import ml_dtypes

import concourse.bass as bass  # noqa: F401  (registers arch tables)
import concourse.mybir as mybir
import concourse.tile as tile
from concourse import bacc
from concourse.bass_utils import run_bass_kernel_spmd

FP8 = mybir.dt.float8e4
F32 = mybir.dt.float32

NCORES = 8
N, H, W, C = 32, 56, 56, 256
HP, WP = H + 2, W + 2           # padded spatial 58x58
NIMG = N // NCORES              # images per core
TC = W // 2                     # 28 tile columns (output col pairs)
NINST = 15                      # weight instances (15 tap-streams)
OPIX = H * W                    # 3136 outputs per (img, chunk) in packed order

# output row groups: 16,16,16,8 rows; FD = rows*28 <= 448 (PSUM bank 512 f32)
GROUP_ROWS = (16, 16, 16, 8)
GROUP_ROW0 = (0, 16, 32, 48)
FDMAX = 16 * TC                 # 448

# V planes split into 2 row-pieces per image so early groups start sooner.
# piece 0: padded rows 0..33 (groups 0,1 need rows 0..33)
# piece 1: padded rows 32..57 (groups 2,3 need rows 32..57)
P0_ROWS, P0_ROW0 = 34, 0
P1_ROWS, P1_ROW0 = 26, 32
P0_LEN = P0_ROWS * TC           # 952
P1_LEN = P1_ROWS * TC           # 728
P0_FREE = 960                   # padded to mult of 16 (DoubleRow step rule)
P1_FREE = 736

# instance tables: which V plane each weight instance streams, which PSUM
# bank it accumulates into (see module docstring), and its vertical tap.
INST_COMP = [1, 1, 1, 0, 0, 0, 2, 2, 2, 2, 2, 2, 3, 3, 3]
INST_BANK = [0, 0, 0, 1, 1, 1, 1, 1, 1, 2, 2, 2, 2, 2, 2]
INST_TAP = [0, 1, 2, 0, 1, 2, 0, 1, 2, 0, 1, 2, 0, 1, 2]

_NC = None


def _build_nc():
    nc = bacc.Bacc("TRN2", target_bir_lowering=False, debug=False)

    x0 = nc.dram_tensor("x0", [NIMG, 2, 4, 128, P0_FREE], FP8, kind="ExternalInput")
    x1 = nc.dram_tensor("x1", [NIMG, 2, 4, 128, P1_FREE], FP8, kind="ExternalInput")
    wb = nc.dram_tensor("wb", [128, NINST, 2, C], FP8, kind="ExternalInput")
    kb = nc.dram_tensor("kb", [NIMG, 1, OPIX], F32, kind="ExternalInput")
    ab = nc.dram_tensor("ab", [128, 2], F32, kind="ExternalInput")
    bb = nc.dram_tensor("bb", [128, 2], F32, kind="ExternalInput")
    ob = nc.dram_tensor("ob", [NIMG, 2, 128, OPIX], F32, kind="ExternalOutput")

    with tile.TileContext(nc) as tc:
        with (
            tc.tile_pool(name="wp", bufs=1) as wp,
            tc.tile_pool(name="xp", bufs=2) as xp,
            tc.tile_pool(name="kp", bufs=2) as kp,
            tc.tile_pool(name="ep", bufs=3) as ep,
            tc.tile_pool(name="op", bufs=4) as op,
            tc.tile_pool(name="ps", bufs=6, space="PSUM") as ps,
        ):
            def dma_x(img):
                x_0 = xp.tile([128, 2, 4, P0_FREE], FP8, tag="x0")
                nc.sync.dma_start(
                    x_0[:, :, :, :P0_LEN],
                    x0[img, :, :, :, :P0_LEN].rearrange("k c p f -> p k c f"),
                )
                x_1 = xp.tile([128, 2, 4, P1_FREE], FP8, tag="x1")
                nc.sync.dma_start(
                    x_1[:, :, :, :P1_LEN],
                    x1[img, :, :, :, :P1_LEN].rearrange("k c p f -> p k c f"),
                )
                return x_0, x_1

            def dma_k(img):
                k1_sb = kp.tile([1, OPIX], F32, tag="k1")
                nc.sync.dma_start(k1_sb[:], kb[img])
                k_sb = kp.tile([128, OPIX], F32, tag="kbig")
                nc.gpsimd.partition_broadcast(k_sb[:], k1_sb[:])
                return k_sb

            w_sb = wp.tile([128, NINST, 2, C], FP8)
            nc.sync.dma_start(w_sb[:], wb[:])
            a_sb = wp.tile([128, 2], F32, tag="a")
            nc.sync.dma_start(a_sb[:], ab[:])
            b_sb = wp.tile([128, 2], F32, tag="b")
            nc.sync.dma_start(b_sb[:], bb[:])

            # warm the PE clock (HAM) with matmuls on a memset scratch tile
            # while the first image's V planes are still in flight
            scr = wp.tile([128, 2, FDMAX], FP8, tag="scr")
            nc.vector.memset(scr[:], 0)
            warm_ps = ps.tile([128, FDMAX], F32, tag="pt", bufs=1)
            for _ in range(18):
                nc.tensor.matmul(
                    warm_ps[:],
                    scr[:, :, 0:128],
                    scr[:],
                    start=True,
                    stop=True,
                    perf_mode=mybir.MatmulPerfMode.DoubleRow,
                )

            for img in range(NIMG):
                x_0, x_1 = dma_x(img)
                k_sb = dma_k(img)

                for c in range(2):
                    for g in range(4):
                        rows = GROUP_ROWS[g]
                        fd = rows * TC
                        src = x_0 if g < 2 else x_1
                        base = GROUP_ROW0[g] - (P0_ROW0 if g < 2 else P1_ROW0)

                        banks = [
                            ps.tile(
                                [128, FDMAX],
                                F32,
                                name=f"bank{j}",
                                tag=f"bank{j}",
                                bufs=2,
                            )
                            for j in range(3)
                        ]
                        started = [False, False, False]
                        for i in range(NINST):
                            bk = INST_BANK[i]
                            off = (base + INST_TAP[i]) * TC
                            last = (
                                i == 2 if bk == 0
                                else i == 8 if bk == 1
                                else i == 14
                            )
                            nc.tensor.matmul(
                                banks[bk][:, :fd],
                                w_sb[:, i, :, c * 128 : (c + 1) * 128],
                                src[:, :, INST_COMP[i], off : off + fd],
                                start=not started[bk],
                                stop=last,
                                perf_mode=mybir.MatmulPerfMode.DoubleRow,
                            )
                            started[bk] = True

                        # epilogue: y_even = B1+B2, y_odd = B1+B3, then
                        # *K*alpha (fused) and +bias
                        goff = GROUP_ROW0[g] * W
                        sb = ep.tile([128, FDMAX], F32, tag="sb")
                        nc.scalar.copy(sb[:, :fd], banks[0][:, :fd])
                        o_sb = op.tile([128, 2 * FDMAX], F32, tag="o")
                        for par, bank in ((0, banks[1]), (1, banks[2])):
                            t = ep.tile([128, FDMAX], F32, tag=f"t{par}")
                            nc.vector.scalar_tensor_tensor(
                                t[:, :fd],
                                bank[:, :fd],
                                0.0,
                                sb[:, :fd],
                                mybir.AluOpType.bypass,
                                mybir.AluOpType.add,
                            )
                            y = ep.tile([128, FDMAX], F32, tag=f"y{par}")
                            nc.vector.scalar_tensor_tensor(
                                y[:, :fd],
                                t[:, :fd],
                                a_sb[:, c : c + 1],
                                k_sb[:, goff + par * fd : goff + (par + 1) * fd],
                                mybir.AluOpType.mult,
                                mybir.AluOpType.mult,
                            )
                            nc.scalar.activation(
                                o_sb[:, par * fd : (par + 1) * fd],
                                y[:, :fd],
                                mybir.ActivationFunctionType.Identity,
                                bias=b_sb[:, c : c + 1],
                                scale=1.0,
                            )
                        nc.sync.dma_start(
                            ob[img, c, :, goff : goff + 2 * fd],
                            o_sb[:, : 2 * fd],
                        )

    nc.compile()
    return nc


def get_nc():
    global _NC
    if _NC is None:
        _NC = _build_nc()
    return _NC


def prep_inputs(x, kernel, bias):
    """Host-side prep: binarize, pad, Winograd-transform; per-core in_maps."""
    np_fp8 = mybir.dt.np(FP8)
    xp = np.pad(x, ((0, 0), (1, 1), (1, 1), (0, 0)))
    binx = np.where(xp > 0, np.float32(1.0), np.float32(-1.0))
    b = np.ascontiguousarray(binx.transpose(0, 3, 1, 2))  # (N, 256, 58, 58)
    d0 = b[..., 0:56:2]
    d1 = b[..., 1:57:2]
    d2 = b[..., 2:58:2]
    d3 = b[..., 3::2]
    # V planes (N, 256, 4, 58, 28), values in {0, +-2}: exact in fp8
    V = np.stack([d0 - d2, d1 + d2, d2 - d1, d1 - d3], axis=2).astype(np_fp8)
    V = V.reshape(N, 2, 128, 4, HP * TC)  # ci = ci_hi*128 + ci_lo

    x0_all = np.zeros((N, 2, 4, 128, P0_FREE), dtype=np_fp8)
    x0_all[..., :P0_LEN] = V[:, :, :, :, :P0_LEN].transpose(0, 1, 3, 2, 4)
    x1_all = np.zeros((N, 2, 4, 128, P1_FREE), dtype=np_fp8)
    x1_all[..., :P1_LEN] = V[
        :, :, :, :, P1_ROW0 * TC : P1_ROW0 * TC + P1_LEN
    ].transpose(0, 1, 3, 2, 4)

    # K = avgpool3x3(beta), packed per group as [g, parity, row, tilecol]
    beta = np.abs(xp).mean(axis=3)
    ks = beta[:, 0:H, :] + beta[:, 1 : H + 1, :] + beta[:, 2 : H + 2, :]
    K = (ks[:, :, 0:W] + ks[:, :, 1 : W + 1] + ks[:, :, 2 : W + 2]) / np.float32(9.0)
    K_pack = np.empty((N, 1, OPIX), dtype=np.float32)
    for g in range(4):
        r0, rows = GROUP_ROW0[g], GROUP_ROWS[g]
        seg = K[:, r0 : r0 + rows, :].reshape(N, rows, TC, 2)  # [r, c, par]
        K_pack[:, 0, r0 * W : (r0 + rows) * W] = seg.transpose(0, 3, 1, 2).reshape(
            N, rows * W
        )

    # Winograd weight instances (15): see module docstring
    g3 = np.where(kernel > 0, np.float32(1.0), np.float32(-1.0))  # (3,3,256,256)
    U1 = g3[:, 0]
    U2 = (g3[:, 0] + g3[:, 1] + g3[:, 2]) / np.float32(2.0)
    U3 = (g3[:, 0] - g3[:, 1] + g3[:, 2]) / np.float32(2.0)
    U4 = g3[:, 2]
    # instance list: (U comp, sign) aligned with INST_* tables
    inst = [U2, U2, U2, U1, U1, U1, U3, U3, U3, -U3, -U3, -U3, -U4, -U4, -U4]
    wb = np.empty((128, NINST, 2, C), dtype=np_fp8)
    for i in range(NINST):
        u = inst[i][INST_TAP[i]].reshape(2, 128, C)  # (ci_hi, ci_lo, co)
        wb[:, i] = u.transpose(1, 0, 2).astype(np_fp8)

    alpha = np.abs(kernel).mean(axis=(0, 1, 2)).astype(np.float32)
    ab = np.ascontiguousarray(alpha.reshape(2, 128).T)
    bb = np.ascontiguousarray(bias.astype(np.float32).reshape(2, 128).T)

    in_maps = []
    for core in range(NCORES):
        sl = slice(core * NIMG, (core + 1) * NIMG)
        in_maps.append(
            {
                "x0": np.ascontiguousarray(x0_all[sl]),
                "x1": np.ascontiguousarray(x1_all[sl]),
                "kb": np.ascontiguousarray(K_pack[sl]),
                "wb": wb,
                "ab": ab,
                "bb": bb,
            }
        )
    return in_maps


def assemble_output(results):
    """(8 cores x (NIMG, 2, 128, OPIX)) -> (N, H, W, C) f32."""
    ot = np.concatenate([r["ob"] for r in results], axis=0)  # (N, 2, 128, OPIX)
    out = np.empty((N, H, W, C), dtype=np.float32)
    for g in range(4):
        r0, rows = GROUP_ROW0[g], GROUP_ROWS[g]
        seg = ot[:, :, :, r0 * W : (r0 + rows) * W].reshape(
            N, 2, 128, 2, rows, TC
        )  # [n, chunk, colo, par, r, c]
        out[:, r0 : r0 + rows] = (
            seg.transpose(0, 4, 5, 3, 1, 2).reshape(N, rows, W, C)
        )
    return out


def kernel(x, kernel, bias, _trace=False):
    nc = get_nc()
    in_maps = prep_inputs(x, kernel, bias)
    res = run_bass_kernel_spmd(
        nc, in_maps, core_ids=list(range(NCORES)), trace=_trace
    )
    out = assemble_output(res.results)
    if _trace:
        return out, res
    return out
